# revision 8
# baseline (speedup 1.0000x reference)
"""Trainium2 Bass kernel for nn_CrossAttention (B=4, C=256, H=W=48, heads=4).

Sharding: 8 cores = 4 batches x 2 halves. Queries split by row-half per
core; raw k/v split by row-half and exchanged on-device via a pair
AllGather; the folded 1x1-conv weights are sharded 1/8 per core and
8-way AllGathered. All per-core host data is packed into one bf16
tensor plus one small f32 tensor so each call stages two parameters.
The positional depthwise 3x3 conv runs on the vector engine as nine
shifted multiply-accumulates. The bf16 output is 8-way AllGathered on
device so the host fetches a single core's (full) output shard.
"""

import numpy as np
import ml_dtypes

import concourse.bass as bass
import concourse.mybir as mybir
import concourse.tile as tile
from concourse import bacc

F32 = mybir.dt.float32
BF16 = mybir.dt.bfloat16
F8 = mybir.dt.float8e4

C = 256
H = W = 48
NK = H * W            # 2304 keys
KC = NK // 128        # 18 key chunks
HEADS = 4
HD = 64
ROWS_HALF = 24        # rows per core
NQ = ROWS_HALF * W    # 1152 query positions per core
QS = 384              # query slice (8 rows)
NQS = NQ // QS        # 3 slices
QROWS = ROWS_HALF + 2  # 26 rows incl halo
NQH = QROWS * W       # 1248
NKH = NK // 2         # 1152 keys per core before exchange
EPS = 1e-5

# big (bf16) input layout, element offsets
OW = 0
NW = 4 * C * C // 8           # 32768 (1/8 of the four weight matrices)
OH = OW + NW
NH = NQH
OT = OH + NH
NT = 2 * 128
XLEN = OT + NT                # 34272

# big8 (fp8) input layout
NKV = 2 * C * NKH             # 589824 (raw k/v own half)
O8Q = NKV
NQX = C * NQH                 # 319488 (raw q rows incl halo)
X8LEN = NKV + NQX             # 909312

# small (f32) input layout
SK = 0                        # tk  [2,128]
SV = SK + 256                 # tv  [264]  (4 heads x 66, bias in vf layout)
SP = SV + 264                 # tp  [2,128]
SJ = SP + 256                 # tj  [2,128]
SW9 = SJ + 256                # w9  [2,9,128]
SLEN = SW9 + 2304             # 3336

_CACHE = {}


def _build():
    nc = bacc.Bacc("TRN2", target_bir_lowering=False, num_devices=8)
    big = nc.dram_tensor("big", [XLEN], BF16, kind="ExternalInput")
    small = nc.dram_tensor("small", [SLEN], F32, kind="ExternalInput")
    big8 = nc.dram_tensor("big8", [X8LEN], F8, kind="ExternalInput")
    o = nc.dram_tensor("o", [1, C, NQ], BF16, kind="ExternalOutput")

    with tile.TileContext(nc) as tc:
        with (
            tc.tile_pool(name="dram", bufs=1, space="DRAM") as dram,
            tc.tile_pool(name="wp", bufs=1) as wp,
            tc.tile_pool(name="inp", bufs=1) as inp,
            tc.tile_pool(name="feat", bufs=1) as feat,
            tc.tile_pool(name="vfp", bufs=18) as vfp,
            tc.tile_pool(name="et", bufs=40) as etp,
            tc.tile_pool(name="small", bufs=3) as smp,
            tc.tile_pool(name="ps_s", bufs=2, space="PSUM") as ps_s,
            tc.tile_pool(name="ps_w", bufs=2, space="PSUM") as ps_w,
        ):
            # ---- weight AllGather: 1/8 slice per core -> full four matrices ----
            win = dram.tile([1, NW], BF16, tag="win")
            nc.gpsimd.dma_start(
                out=win[:], in_=big[OW:OW + NW].rearrange("(x n) -> x n", x=1))
            wall = dram.tile([4, 2, 128, C], BF16, tag="wall", addr_space="Shared")
            nc.gpsimd.collective_compute(
                "AllGather", mybir.AluOpType.bypass,
                replica_groups=[[0, 1, 2, 3, 4, 5, 6, 7]],
                ins=[win.opt()], outs=[wall.opt()],
            )
            # ---- k/v pair AllGather: own key half -> both halves (fp8) ----
            kvin = dram.tile([2, 2, 128, NKH], F8, tag="kvin")
            nc.gpsimd.dma_start(
                out=kvin[:],
                in_=big8[0:NKV].rearrange("(t a p n) -> t a p n", t=2, a=2, p=128))
            kvout = dram.tile([2, 2, 2, 128, NKH], F8, tag="kvout")
            nc.gpsimd.collective_compute(
                "AllGather", mybir.AluOpType.bypass,
                replica_groups=[[0, 1], [2, 3], [4, 5], [6, 7]],
                ins=[kvin.opt()], outs=[kvout.opt()],
            )

            # ---- inputs / weights to SBUF ----
            q8 = inp.tile([128, 2, NQH], F8, tag="q8")
            nc.sync.dma_start(
                out=q8[:], in_=big8[O8Q:O8Q + NQX].rearrange("(a p n) -> p a n", p=128, n=NQH))
            q_sb = inp.tile([128, 2, NQH], BF16, tag="q")
            nc.vector.tensor_copy(q_sb[:], q8[:])
            hq_sb = wp.tile([1, NQH], BF16, tag="hq")
            nc.sync.dma_start(
                out=hq_sb[:], in_=big[OH:OH + NH].rearrange("(x n) -> x n", x=1))
            tq_sb = wp.tile([1, 2, 128], BF16, tag="tq")
            nc.sync.dma_start(
                out=tq_sb[:], in_=big[OT:OT + NT].rearrange("(x a n) -> x a n", x=1, a=2))

            w_q = wp.tile([128, 2, C], BF16, tag="wq")
            w_k = wp.tile([128, 2, C], BF16, tag="wk")
            w_v = wp.tile([128, 2, C], BF16, tag="wv")
            w_p = wp.tile([128, 2, C], BF16, tag="wpj")
            for wi, t in enumerate((w_q, w_k, w_v, w_p)):
                nc.sync.dma_start(out=t[:], in_=wall[wi].rearrange("a p n -> p a n"))

            k8 = inp.tile([128, 2, NK], F8, tag="k8")
            v8 = inp.tile([128, 2, NK], F8, tag="v8")
            for hh in range(2):
                nc.sync.dma_start(
                    out=k8[:, :, hh * NKH:(hh + 1) * NKH],
                    in_=kvout[hh, 0].rearrange("a p n -> p a n"))
                nc.sync.dma_start(
                    out=v8[:, :, hh * NKH:(hh + 1) * NKH],
                    in_=kvout[hh, 1].rearrange("a p n -> p a n"))
            k_sb = inp.tile([128, 2, NK], BF16, tag="k")
            v_sb = inp.tile([128, 2, NK], BF16, tag="v")
            nc.vector.tensor_copy(k_sb[:], k8[:])
            nc.vector.tensor_copy(v_sb[:], v8[:])

            tk_sb = wp.tile([128, 2, 1], F32, tag="tk")
            nc.sync.dma_start(
                out=tk_sb[:], in_=small[SK:SK + 256].rearrange("(a p x) -> p a x", p=128, x=1))
            tp_sb = wp.tile([128, 2, 1], F32, tag="tp")
            nc.sync.dma_start(
                out=tp_sb[:], in_=small[SP:SP + 256].rearrange("(a p x) -> p a x", p=128, x=1))
            tj_sb = wp.tile([128, 2, 1], F32, tag="tj")
            nc.sync.dma_start(
                out=tj_sb[:], in_=small[SJ:SJ + 256].rearrange("(a p x) -> p a x", p=128, x=1))
            w9_sb = wp.tile([128, 2, 9], F32, tag="w9")
            nc.sync.dma_start(
                out=w9_sb[:], in_=small[SW9:SW9 + 2304].rearrange("(a t p) -> p a t", a=2, t=9))
            tv1 = wp.tile([1, 264], F32, tag="tv1")
            nc.sync.dma_start(
                out=tv1[:], in_=small[SV:SV + 264].rearrange("(x n) -> x n", x=1))
            tv_sb = wp.tile([128, 264], F32, tag="tv")
            nc.gpsimd.partition_broadcast(tv_sb[:], tv1[:])

            # ---- qf: channel-major query features (scaled), with halo rows ----
            qf = feat.tile([128, 2, NQH], BF16, tag="qf")
            for co in range(2):
                for n0 in range(0, NQH, 512):
                    nn = min(512, NQH - n0)
                    ps = ps_w.tile([128, 512], F32, tag="w")
                    for ci in range(2):
                        nc.tensor.matmul(
                            ps[:, 0:nn],
                            w_q[:, ci, co * 128:(co + 1) * 128],
                            q_sb[:, ci, n0:n0 + nn],
                            start=(ci == 0), stop=False,
                        )
                    # masked bias: qf += tq[c] * hmask[n]  (rank-1)
                    nc.tensor.matmul(
                        ps[:, 0:nn],
                        tq_sb[:, co, :],
                        hq_sb[:, n0:n0 + nn],
                        start=False, stop=True,
                    )
                    nc.vector.tensor_copy(qf[:, co, n0:n0 + nn], ps[:, 0:nn])

            # ---- kf: channel-major key features [128, 2, NK] bf16 ----
            kf = feat.tile([128, 2, NK], BF16, tag="kf")
            for co in range(2):
                for n0 in range(0, NK, 512):
                    nn = min(512, NK - n0)
                    ps = ps_w.tile([128, 512], F32, tag="w")
                    for ci in range(2):
                        nc.tensor.matmul(
                            ps[:, 0:nn],
                            w_k[:, ci, co * 128:(co + 1) * 128],
                            k_sb[:, ci, n0:n0 + nn],
                            start=(ci == 0), stop=(ci == 1),
                        )
                    nc.vector.tensor_scalar(
                        kf[:, co, n0:n0 + nn], ps[:, 0:nn],
                        tk_sb[:, co, :], None, mybir.AluOpType.add,
                    )

            # ---- vf: position-major value features, 18 tiles [128, 4, 66] ----
            # per head h: cols [v(64) | 1 | pad]
            vf = []
            for pc in range(KC):
                vt = vfp.tile([128, 4, 66], BF16, tag="vf")
                nc.vector.memset(vt[:], 1.0)
                ps = ps_w.tile([128, 512], F32, tag="w")
                for ci in range(2):
                    nc.tensor.matmul(
                        ps[:, 0:C],
                        v_sb[:, ci, pc * 128:(pc + 1) * 128],
                        w_v[:, ci, :],
                        start=(ci == 0), stop=(ci == 1),
                    )
                psv = ps[:, 0:C].rearrange("p (h d) -> p h d", h=4)
                tvv = tv_sb[:].rearrange("p (h f) -> p h f", h=4)
                nc.vector.tensor_add(vt[:, :, 0:64], psv[:], tvv[:, :, 0:64])
                vf.append(vt)

            qfr = qf[:].rearrange("p a (r w) -> p a r w", w=W)

            # ---- attention + pe + proj, software-pipelined across q slices:
            # while ACT runs exp for slice si, PE runs AV/pe/proj of si-1.
            def emit_s_group(st, t, h):
                hp, par = h // 2, h % 2
                rs = slice(par * 64, par * 64 + 64)
                s = ps_s.tile([128, 3, 512], F32, tag="s")
                for i in range(3):
                    kc = t * 3 + i
                    nc.tensor.matmul(
                        s[:, i, 0:QS],
                        kf[rs, hp, kc * 128:(kc + 1) * 128],
                        qf[rs, hp, st["q0"]:st["q0"] + QS],
                        start=True, stop=True,
                    )
                et = etp.tile([128, 3, QS], BF16, tag="et")
                nc.scalar.activation(et[:], s[:, :, 0:QS],
                                     mybir.ActivationFunctionType.Exp)
                st["ets"][t][h] = et

            def emit_av_head(st, h):
                y = ps_w.tile([128, 512], F32, tag="w")
                for t in range(6):
                    for i in range(3):
                        kc = t * 3 + i
                        nc.tensor.matmul(
                            y[0:65, 0:QS], vf[kc][:, h, 0:65],
                            st["ets"][t][h][:, i, :],
                            start=(kc == 0), stop=(kc == KC - 1),
                        )
                st["ys"][h] = y

            def emit_norm(st, pair):
                ys = [st["ys"][pair * 2], st["ys"][pair * 2 + 1]]
                ynt = smp.tile([128, QS], BF16, tag="yn")
                rr = smp.tile([1, 2, QS], F32, tag="rr")
                rq = smp.tile([128, 2, QS], F32, tag="rq")
                for par in range(2):
                    nc.vector.reciprocal(rr[:, par, :], ys[par][64:65, 0:QS])
                nc.gpsimd.partition_broadcast(rq[:], rr[:])
                nc.vector.tensor_mul(ynt[0:64, :], ys[0][0:64, 0:QS], rq[0:64, 0, :])
                nc.vector.tensor_mul(ynt[64:128, :], ys[1][0:64, 0:QS], rq[64:128, 1, :])
                st["yn"][pair] = ynt

            # pe taps ordered so the first writes the full width (dj == 0)
            PE_TAPS = [(-1, 0), (-1, -1), (-1, 1), (0, -1), (0, 0), (0, 1),
                       (1, -1), (1, 0), (1, 1)]

            def emit_tail(st):
                r0, si = st["r0"], st["si"]
                yt = [None, None]
                for ch in range(2):
                    pet = smp.tile([128, QS], F32, tag="pe")
                    pev = pet[:].rearrange("p (r w) -> p r w", w=W)
                    for idx, (di, dj) in enumerate(PE_TAPS):
                        ti = (di + 1) * 3 + (dj + 1)
                        j0o, j0i = max(0, -dj), max(0, dj)
                        ncol = W - abs(dj)
                        src = qfr[:, ch, r0 + 1 + di:r0 + 9 + di, j0i:j0i + ncol]
                        if idx == 0:
                            nc.vector.tensor_scalar(
                                pev[:, :, j0o:j0o + ncol], src,
                                w9_sb[:, ch, ti:ti + 1], None, mybir.AluOpType.mult,
                            )
                        else:
                            nc.vector.scalar_tensor_tensor(
                                out=pev[:, :, j0o:j0o + ncol], in0=src,
                                scalar=w9_sb[:, ch, ti:ti + 1],
                                in1=pev[:, :, j0o:j0o + ncol],
                                op0=mybir.AluOpType.mult, op1=mybir.AluOpType.add,
                            )
                    ytt = smp.tile([128, QS], BF16, tag="yt")
                    nc.vector.scalar_tensor_tensor(
                        out=ytt[:], in0=pet[:], scalar=tp_sb[:, ch, :],
                        in1=st["yn"][ch][:], op0=mybir.AluOpType.add,
                        op1=mybir.AluOpType.add,
                    )
                    yt[ch] = ytt
                ob = smp.tile([128, 2, QS], BF16, tag="ob")
                for co in range(2):
                    pj = ps_w.tile([128, 512], F32, tag="w")
                    for ci in range(2):
                        nc.tensor.matmul(
                            pj[:, 0:QS],
                            w_p[:, ci, co * 128:(co + 1) * 128],
                            yt[ci][:],
                            start=(ci == 0), stop=(ci == 1),
                        )
                    nc.vector.tensor_scalar(
                        ob[:, co, :], pj[:, 0:QS], tj_sb[:, co, :], None,
                        mybir.AluOpType.add,
                    )
                nc.sync.dma_start(
                    out=o[0].rearrange("(a p) n -> p a n", p=128)[:, :, si * QS:(si + 1) * QS],
                    in_=ob[:],
                )

            FIRE = {4: lambda st: emit_av_head(st, 0),
                    8: lambda st: emit_av_head(st, 1),
                    12: lambda st: emit_norm(st, 0),
                    16: lambda st: emit_av_head(st, 2),
                    20: lambda st: emit_av_head(st, 3),
                    24: lambda st: emit_norm(st, 1)}

            prev = None
            for si in range(NQS + 1):
                cur = None
                if si < NQS:
                    cur = {"si": si, "q0": 48 + si * QS, "r0": si * (QS // W),
                           "ets": [[None] * HEADS for _ in range(6)],
                           "ys": [None] * 4, "yn": [None, None]}
                    g = 0
                    for t in range(6):
                        for h in range(HEADS):
                            emit_s_group(cur, t, h)
                            g += 1
                            if prev is not None and g in FIRE:
                                FIRE[g](prev)
                    if prev is not None:
                        emit_tail(prev)
                else:
                    for g in (4, 8, 12, 16, 20, 24):
                        FIRE[g](prev)
                    emit_tail(prev)
                prev = cur
    nc.compile()
    return nc


def _prep(inputs):
    """Host-side: fold BN into weights, pack per-core staged buffers."""
    f64 = np.float64
    bf = ml_dtypes.bfloat16

    def fold(w, g, b, m, v):
        s = g.astype(f64) / np.sqrt(v.astype(f64) + EPS)
        return w.astype(f64) * s[:, None], b.astype(f64) - m.astype(f64) * s

    wq, tq = fold(inputs["wq_w"], inputs["wq_g"], inputs["wq_b"], inputs["wq_m"], inputs["wq_v"])
    wk, tk = fold(inputs["wk_w"], inputs["wk_g"], inputs["wk_b"], inputs["wk_m"], inputs["wk_v"])
    wv, tv = fold(inputs["wv_w"], inputs["wv_g"], inputs["wv_b"], inputs["wv_m"], inputs["wv_v"])
    wp, tj = fold(inputs["proj_w"], inputs["proj_g"], inputs["proj_b"], inputs["proj_m"], inputs["proj_v"])
    scale = 1.0 / np.sqrt(HD)
    wq, tq = wq * scale, tq * scale
    s_pe = inputs["pe_g"].astype(f64) / np.sqrt(inputs["pe_v"].astype(f64) + EPS)
    tp = inputs["pe_b"].astype(f64) - inputs["pe_m"].astype(f64) * s_pe
    w9 = inputs["pe_w"].astype(f64).reshape(C, 9) * s_pe[:, None] / scale  # pe sees unscaled qf

    # four weight matrices, transposed, flat in [4, 2, 128, C] order
    w4 = np.empty((4, C, C), dtype=bf)
    for i, m in enumerate((wq, wk, wv, wp)):
        w4[i] = m.T.astype(bf)
    w4f = w4.reshape(4 * C * C)

    # small f32 buffer (identical on every core)
    small = np.zeros(SLEN, dtype=np.float32)
    small[SK:SK + 256] = tk.astype(np.float32)
    tvv = tv.astype(np.float32).reshape(4, 64)
    svv = small[SV:SV + 264].reshape(4, 66)
    svv[:, 0:64] = tvv
    small[SP:SP + 256] = tp.astype(np.float32)
    small[SJ:SJ + 256] = tj.astype(np.float32)
    # w9 packed (a, tap, p)
    small[SW9:SW9 + 2304] = (
        w9.reshape(2, 128, 9).transpose(0, 2, 1).astype(np.float32).reshape(-1))

    f8 = ml_dtypes.float8_e4m3
    if "big" not in _CACHE:
        _CACHE["big"] = np.empty((8, XLEN), dtype=bf)
        _CACHE["small"] = np.empty((8, SLEN), dtype=np.float32)
        _CACHE["big8"] = np.empty((8, X8LEN), dtype=f8)
    bigb = _CACHE["big"]
    smallb = _CACHE["small"]
    big8b = _CACHE["big8"]
    smallb[:] = small[None, :]

    q = inputs["q"].astype(f8).reshape(4, C, H, W)
    k = inputs["k"].astype(f8).reshape(4, C, H, W)
    v = inputs["v"].astype(f8).reshape(4, C, H, W)
    tqb = tq.astype(bf)

    for c in range(8):
        b, half = c // 2, c % 2
        r0 = half * ROWS_HALF
        qx = big8b[c, O8Q:O8Q + NQX].reshape(C, QROWS, W)
        hm = np.zeros((QROWS,), dtype=bf)
        lo, hi = max(0, r0 - 1), min(H, r0 + ROWS_HALF + 1)
        a0 = lo - (r0 - 1)
        if a0 > 0:
            qx[:, 0:a0] = 0
        if a0 + (hi - lo) < QROWS:
            qx[:, a0 + (hi - lo):] = 0
        qx[:, a0:a0 + (hi - lo)] = q[b, :, lo:hi]
        hm[a0:a0 + (hi - lo)] = 1
        kv = big8b[c, 0:NKV].reshape(2, C, NKH)
        kv[0] = k[b, :, r0:r0 + ROWS_HALF].reshape(C, NKH)
        kv[1] = v[b, :, r0:r0 + ROWS_HALF].reshape(C, NKH)
        bigb[c, OW:OW + NW] = w4f[c * NW:(c + 1) * NW]
        bigb[c, OH:OH + NH] = np.repeat(hm, W)
        bigb[c, OT:OT + NT] = tqb

    # stage the packed buffers on-device (sharded over the 8 cores) so
    # repeated run_cores calls don't re-ship inputs over the tunnel
    try:
        dev = _device_put(bigb, smallb, big8b)
    except Exception as e:
        import sys
        print(f"kernel: device_put failed ({type(e).__name__}: {e}); "
              f"staying host-side", file=sys.stderr)
        dev = None
    return bigb, smallb, big8b, dev


def _get_mesh():
    if "mesh" not in _CACHE:
        import jax
        from jax.sharding import Mesh
        _CACHE["mesh"] = Mesh(np.asarray(jax.devices()[:8]), ("core",))
    return _CACHE["mesh"]


def _device_put(bigb, smallb, big8b):
    import jax
    from jax.sharding import NamedSharding, PartitionSpec
    sh = NamedSharding(_get_mesh(), PartitionSpec("core"))
    arrs = (
        jax.device_put(bigb.reshape(-1), sh),
        jax.device_put(smallb.reshape(-1), sh),
        jax.device_put(big8b.reshape(-1), sh),
    )
    jax.block_until_ready(arrs)
    return arrs


def _get_nc():
    if "nc" not in _CACHE:
        _CACHE["nc"] = _build()
    return _CACHE["nc"]


def _get_runner():
    if "runner" in _CACHE:
        return _CACHE["runner"]
    import jax
    from jax.sharding import Mesh, PartitionSpec
    from jax.experimental.shard_map import shard_map
    from concourse import bass2jax

    nc = _get_nc()
    bass2jax.install_neuronx_cc_hook()
    out_aval = jax.core.ShapedArray((1, C, NQ), ml_dtypes.bfloat16)
    pid_name = nc.partition_id_tensor.name if nc.partition_id_tensor else None
    in_names = ("big", "small", "big8") + ((pid_name,) if pid_name else ())

    def _body(bigv, smallv, big8v):
        operands = [bigv, smallv, big8v]
        if pid_name is not None:
            operands.append(bass2jax.partition_id_tensor())
        outs = bass2jax._bass_exec_p.bind(
            *operands,
            out_avals=(out_aval,),
            in_names=in_names,
            out_names=("o",),
            lowering_input_output_aliases=(),
            sim_require_finite=True,
            sim_require_nnan=True,
            nc=nc,
        )
        return tuple(outs)

    mesh = _get_mesh()
    sharded = jax.jit(
        shard_map(
            _body, mesh=mesh,
            in_specs=(PartitionSpec("core"),) * 3,
            out_specs=(PartitionSpec("core"),),
            check_rep=False,
        ),
        keep_unused=True,
    )
    _CACHE["runner"] = sharded
    return sharded


def _run_fallback(big, small, big8):
    from concourse.bass_utils import run_bass_kernel_spmd
    in_maps = [{"big": big[c], "small": small[c], "big8": big8[c]}
               for c in range(8)]
    res = run_bass_kernel_spmd(_get_nc(), in_maps, core_ids=list(range(8)))
    return np.stack([res.results[c]["o"][0] for c in range(8)])


def run_cores(bufs):
    big, small, big8, dev = bufs
    if "runner_failed" in _CACHE or dev is None:
        return _run_fallback(big, small, big8)
    try:
        sharded = _get_runner()
        out, = sharded(*dev)
        out.block_until_ready()
        return out
    except Exception as e:
        import sys
        print(f"kernel: jit runner failed ({type(e).__name__}: {e}); "
              f"using spmd fallback", file=sys.stderr)
        _CACHE["runner_failed"] = True
        return _run_fallback(big, small, big8)


def assemble(out):
    # out: [8, C, NQ] bf16 (each core's half-batch output shard)
    o8 = np.asarray(out).astype(np.float32).reshape(4, 2, C, ROWS_HALF, W)
    return o8.transpose(0, 2, 1, 3, 4).reshape(4, C, H, W).copy()


def kernel(**inputs):
    bufs = _prep(inputs)
    out = run_cores(bufs)
    return assemble(out)



# revision 10
# speedup vs baseline: 1.0273x; 1.0273x over previous
"""Trainium2 Bass kernel for nn_CrossAttention (B=4, C=256, H=W=48, heads=4).

Sharding: 8 cores = 4 batches x 2 query-row halves. Fully collective-free:
the host stages the folded 1x1-conv weights (replicated) plus the full
raw k/v (fp8) and the core's q row-half (with 1-row halo) per core, so
every core runs an independent local pipeline. fp8 inputs feed the
feature matmuls directly (mixed fp8 x bf16 is supported by the PE).

BN folding tricks: the k-projection bias adds a per-query constant to
every logit row, so softmax cancels it -- dropped. The v-projection bias
adds a per-channel constant to the normalized attention output, so it is
folded into the proj bias on the host (tj' = tj + Wp @ tv).

The positional depthwise 3x3 conv runs on the gpsimd engine (nine
shifted multiply-accumulates), keeping the vector engine free for
PSUM evacuation and softmax normalization; exp runs on the scalar
engine, pipelined across query slices against the PE's S/AV matmuls.
"""

import numpy as np
import ml_dtypes

import concourse.bass as bass
import concourse.mybir as mybir
import concourse.tile as tile
from concourse import bacc

F32 = mybir.dt.float32
BF16 = mybir.dt.bfloat16
F8 = mybir.dt.float8e4

C = 256
H = W = 48
NK = H * W            # 2304 keys
KC = NK // 128        # 18 key chunks
HEADS = 4
HD = 64
ROWS_HALF = 24        # rows per core
NQ = ROWS_HALF * W    # 1152 query positions per core
QS = 384              # query slice (8 rows)
NQS = NQ // QS        # 3 slices
QROWS = ROWS_HALF + 2  # 26 rows incl halo
NQH = QROWS * W       # 1248
EPS = 1e-5

# big (bf16) input layout, element offsets
OW = 0
NW = 4 * C * C                # 262144 (four weight matrices, replicated)
OH = OW + NW
NH = NQH
OT = OH + NH
NT = 2 * 128
XLEN = OT + NT                # 263648

# big8 (fp8) input layout
O8K = 0
NKV1 = C * NK                 # 589824
O8V = O8K + NKV1
O8Q = O8V + NKV1
NQX = C * NQH                 # 319488 (raw q rows incl halo)
X8LEN = O8Q + NQX             # 1499136

# small (f32) input layout
SP = 0                        # tp  [2,128]
SJ = SP + 256                 # tj' [2,128]  (tj + Wp @ tv folded)
SW9 = SJ + 256                # w9  [2,9,128]
SLEN = SW9 + 2304             # 2816

_CACHE = {}


def _build():
    nc = bacc.Bacc("TRN2", target_bir_lowering=False, num_devices=8)
    big = nc.dram_tensor("big", [XLEN], BF16, kind="ExternalInput")
    small = nc.dram_tensor("small", [SLEN], F32, kind="ExternalInput")
    big8 = nc.dram_tensor("big8", [X8LEN], F8, kind="ExternalInput")
    o = nc.dram_tensor("o", [1, C, NQ], BF16, kind="ExternalOutput")

    with tile.TileContext(nc) as tc:
        with (
            tc.tile_pool(name="wp", bufs=1) as wp,
            tc.tile_pool(name="inp", bufs=1) as inp,
            tc.tile_pool(name="feat", bufs=1) as feat,
            tc.tile_pool(name="vfp", bufs=18) as vfp,
            tc.tile_pool(name="et", bufs=40) as etp,
            tc.tile_pool(name="small", bufs=3) as smp,
            tc.tile_pool(name="ps_s", bufs=2, space="PSUM") as ps_s,
            tc.tile_pool(name="ps_w", bufs=2, space="PSUM") as ps_w,
        ):
            # ---- weights / params to SBUF (replicated, no collectives) ----
            w_all = wp.tile([128, 4, 2, C], BF16, tag="wall")
            nc.sync.dma_start(
                out=w_all[:],
                in_=big[OW:OW + NW].rearrange("(w a p n) -> p w a n", w=4, a=2, p=128))
            w_q, w_k, w_v, w_p = (w_all[:, i] for i in range(4))

            hq_sb = wp.tile([1, NQH], BF16, tag="hq")
            nc.sync.dma_start(
                out=hq_sb[:], in_=big[OH:OH + NH].rearrange("(x n) -> x n", x=1))
            tq_sb = wp.tile([1, 2, 128], BF16, tag="tq")
            nc.sync.dma_start(
                out=tq_sb[:], in_=big[OT:OT + NT].rearrange("(x a n) -> x a n", x=1, a=2))

            tp_sb = wp.tile([128, 2, 1], F32, tag="tp")
            nc.sync.dma_start(
                out=tp_sb[:], in_=small[SP:SP + 256].rearrange("(a p x) -> p a x", p=128, x=1))
            tj_sb = wp.tile([128, 2, 1], F32, tag="tj")
            nc.sync.dma_start(
                out=tj_sb[:], in_=small[SJ:SJ + 256].rearrange("(a p x) -> p a x", p=128, x=1))
            w9_sb = wp.tile([128, 2, 9], F32, tag="w9")
            nc.sync.dma_start(
                out=w9_sb[:], in_=small[SW9:SW9 + 2304].rearrange("(a t p) -> p a t", a=2, t=9))

            # ---- raw inputs to SBUF (fp8, fed to the PE directly) ----
            q8 = inp.tile([128, 2, NQH], F8, tag="q8")
            nc.sync.dma_start(
                out=q8[:], in_=big8[O8Q:O8Q + NQX].rearrange("(a p n) -> p a n", p=128, n=NQH))
            k8 = inp.tile([128, 2, NK], F8, tag="k8")
            nc.sync.dma_start(
                out=k8[:], in_=big8[O8K:O8K + NKV1].rearrange("(a p n) -> p a n", p=128, n=NK))
            v8 = inp.tile([128, 2, NK], F8, tag="v8")
            nc.sync.dma_start(
                out=v8[:], in_=big8[O8V:O8V + NKV1].rearrange("(a p n) -> p a n", p=128, n=NK))

            # ---- qf: channel-major query features (scaled), with halo rows ----
            qf = feat.tile([128, 2, NQH], BF16, tag="qf")
            for co in range(2):
                for n0 in range(0, NQH, 512):
                    nn = min(512, NQH - n0)
                    ps = ps_w.tile([128, 512], F32, tag="w")
                    for ci in range(2):
                        nc.tensor.matmul(
                            ps[:, 0:nn],
                            w_q[:, ci, co * 128:(co + 1) * 128],
                            q8[:, ci, n0:n0 + nn],
                            start=(ci == 0), stop=False,
                        )
                    # masked bias: qf += tq[c] * hmask[n]  (rank-1)
                    nc.tensor.matmul(
                        ps[:, 0:nn],
                        tq_sb[:, co, :],
                        hq_sb[:, n0:n0 + nn],
                        start=False, stop=True,
                    )
                    nc.vector.tensor_copy(qf[:, co, n0:n0 + nn], ps[:, 0:nn])

            # ---- kf: channel-major key features [128, 2, NK] bf16 ----
            # (no bias: it is constant across keys, softmax cancels it)
            kf = feat.tile([128, 2, NK], BF16, tag="kf")
            for co in range(2):
                for n0 in range(0, NK, 512):
                    nn = min(512, NK - n0)
                    ps = ps_w.tile([128, 512], F32, tag="w")
                    for ci in range(2):
                        nc.tensor.matmul(
                            ps[:, 0:nn],
                            w_k[:, ci, co * 128:(co + 1) * 128],
                            k8[:, ci, n0:n0 + nn],
                            start=(ci == 0), stop=(ci == 1),
                        )
                    nc.vector.tensor_copy(kf[:, co, n0:n0 + nn], ps[:, 0:nn])

            # ---- vf: position-major value features, 18 tiles [128, 4, 66] ----
            # per head h: cols [v(64) | 1 | pad]; no bias (folded into tj')
            vf = []
            for pc in range(KC):
                vt = vfp.tile([128, 4, 66], BF16, tag="vf")
                nc.gpsimd.memset(vt[:, :, 64:66], 1.0)
                ps = ps_w.tile([128, 512], F32, tag="w")
                for ci in range(2):
                    nc.tensor.matmul(
                        ps[:, 0:C],
                        v8[:, ci, pc * 128:(pc + 1) * 128],
                        w_v[:, ci, :],
                        start=(ci == 0), stop=(ci == 1),
                    )
                psv = ps[:, 0:C].rearrange("p (h d) -> p h d", h=4)
                nc.vector.tensor_copy(vt[:, :, 0:64], psv[:])
                vf.append(vt)

            qfr = qf[:].rearrange("p a (r w) -> p a r w", w=W)

            # ---- attention + pe + proj, software-pipelined across q slices:
            # while ACT runs exp for slice si, PE runs AV/pe/proj of si-1.
            def emit_s_group(st, t, h):
                hp, par = h // 2, h % 2
                rs = slice(par * 64, par * 64 + 64)
                s = ps_s.tile([128, 3, 512], F32, tag="s")
                for i in range(3):
                    kc = t * 3 + i
                    nc.tensor.matmul(
                        s[:, i, 0:QS],
                        kf[rs, hp, kc * 128:(kc + 1) * 128],
                        qf[rs, hp, st["q0"]:st["q0"] + QS],
                        start=True, stop=True,
                    )
                et = etp.tile([128, 3, QS], BF16, tag="et")
                nc.scalar.activation(et[:], s[:, :, 0:QS],
                                     mybir.ActivationFunctionType.Exp)
                st["ets"][t][h] = et

            def emit_av_head(st, h):
                y = ps_w.tile([128, 512], F32, tag="w")
                for t in range(6):
                    for i in range(3):
                        kc = t * 3 + i
                        nc.tensor.matmul(
                            y[0:65, 0:QS], vf[kc][:, h, 0:65],
                            st["ets"][t][h][:, i, :],
                            start=(kc == 0), stop=(kc == KC - 1),
                        )
                st["ys"][h] = y

            def emit_norm(st, pair):
                ys = [st["ys"][pair * 2], st["ys"][pair * 2 + 1]]
                ynt = smp.tile([128, QS], BF16, tag="yn")
                rr = smp.tile([1, 2, QS], F32, tag="rr")
                rq = smp.tile([128, 2, QS], F32, tag="rq")
                for par in range(2):
                    nc.vector.reciprocal(rr[:, par, :], ys[par][64:65, 0:QS])
                nc.gpsimd.partition_broadcast(rq[:], rr[:])
                nc.vector.tensor_mul(ynt[0:64, :], ys[0][0:64, 0:QS], rq[0:64, 0, :])
                nc.vector.tensor_mul(ynt[64:128, :], ys[1][0:64, 0:QS], rq[64:128, 1, :])
                st["yn"][pair] = ynt

            # pe taps ordered so the first writes the full width (dj == 0)
            PE_TAPS = [(-1, 0), (-1, -1), (-1, 1), (0, -1), (0, 0), (0, 1),
                       (1, -1), (1, 0), (1, 1)]

            def emit_tail(st):
                r0, si = st["r0"], st["si"]
                yt = [None, None]
                for ch in range(2):
                    pet = smp.tile([128, QS], F32, tag="pe")
                    pev = pet[:].rearrange("p (r w) -> p r w", w=W)
                    for idx, (di, dj) in enumerate(PE_TAPS):
                        ti = (di + 1) * 3 + (dj + 1)
                        j0o, j0i = max(0, -dj), max(0, dj)
                        ncol = W - abs(dj)
                        src = qfr[:, ch, r0 + 1 + di:r0 + 9 + di, j0i:j0i + ncol]
                        if idx == 0:
                            nc.vector.tensor_scalar(
                                pev[:, :, j0o:j0o + ncol], src,
                                w9_sb[:, ch, ti:ti + 1], None, mybir.AluOpType.mult,
                            )
                        else:
                            nc.vector.scalar_tensor_tensor(
                                out=pev[:, :, j0o:j0o + ncol], in0=src,
                                scalar=w9_sb[:, ch, ti:ti + 1],
                                in1=pev[:, :, j0o:j0o + ncol],
                                op0=mybir.AluOpType.mult, op1=mybir.AluOpType.add,
                            )
                    ytt = smp.tile([128, QS], BF16, tag="yt")
                    nc.vector.scalar_tensor_tensor(
                        out=ytt[:], in0=pet[:], scalar=tp_sb[:, ch, :],
                        in1=st["yn"][ch][:], op0=mybir.AluOpType.add,
                        op1=mybir.AluOpType.add,
                    )
                    yt[ch] = ytt
                ob = smp.tile([128, 2, QS], BF16, tag="ob")
                for co in range(2):
                    pj = ps_w.tile([128, 512], F32, tag="w")
                    for ci in range(2):
                        nc.tensor.matmul(
                            pj[:, 0:QS],
                            w_p[:, ci, co * 128:(co + 1) * 128],
                            yt[ci][:],
                            start=(ci == 0), stop=(ci == 1),
                        )
                    nc.vector.tensor_scalar(
                        ob[:, co, :], pj[:, 0:QS], tj_sb[:, co, :], None,
                        mybir.AluOpType.add,
                    )
                nc.sync.dma_start(
                    out=o[0].rearrange("(a p) n -> p a n", p=128)[:, :, si * QS:(si + 1) * QS],
                    in_=ob[:],
                )

            FIRE = {4: lambda st: emit_av_head(st, 0),
                    8: lambda st: emit_av_head(st, 1),
                    12: lambda st: emit_norm(st, 0),
                    16: lambda st: emit_av_head(st, 2),
                    20: lambda st: emit_av_head(st, 3),
                    24: lambda st: emit_norm(st, 1)}

            prev = None
            for si in range(NQS + 1):
                cur = None
                if si < NQS:
                    cur = {"si": si, "q0": 48 + si * QS, "r0": si * (QS // W),
                           "ets": [[None] * HEADS for _ in range(6)],
                           "ys": [None] * 4, "yn": [None, None]}
                    g = 0
                    for t in range(6):
                        for h in range(HEADS):
                            emit_s_group(cur, t, h)
                            g += 1
                            if prev is not None and g in FIRE:
                                FIRE[g](prev)
                    if prev is not None:
                        emit_tail(prev)
                else:
                    for g in (4, 8, 12, 16, 20, 24):
                        FIRE[g](prev)
                    emit_tail(prev)
                prev = cur
    nc.compile()
    return nc


def _prep(inputs):
    """Host-side: fold BN into weights, pack per-core staged buffers."""
    f64 = np.float64
    bf = ml_dtypes.bfloat16

    def fold(w, g, b, m, v):
        s = g.astype(f64) / np.sqrt(v.astype(f64) + EPS)
        return w.astype(f64) * s[:, None], b.astype(f64) - m.astype(f64) * s

    wq, tq = fold(inputs["wq_w"], inputs["wq_g"], inputs["wq_b"], inputs["wq_m"], inputs["wq_v"])
    wk, _tk = fold(inputs["wk_w"], inputs["wk_g"], inputs["wk_b"], inputs["wk_m"], inputs["wk_v"])
    wv, tv = fold(inputs["wv_w"], inputs["wv_g"], inputs["wv_b"], inputs["wv_m"], inputs["wv_v"])
    wp, tj = fold(inputs["proj_w"], inputs["proj_g"], inputs["proj_b"], inputs["proj_m"], inputs["proj_v"])
    scale = 1.0 / np.sqrt(HD)
    wq, tq = wq * scale, tq * scale
    tj = tj + wp @ tv  # v-projection bias commutes through proj
    s_pe = inputs["pe_g"].astype(f64) / np.sqrt(inputs["pe_v"].astype(f64) + EPS)
    tp = inputs["pe_b"].astype(f64) - inputs["pe_m"].astype(f64) * s_pe
    w9 = inputs["pe_w"].astype(f64).reshape(C, 9) * s_pe[:, None] / scale  # pe sees unscaled qf

    # four weight matrices, transposed, flat in [4, 2, 128, C] order
    w4 = np.empty((4, C, C), dtype=bf)
    for i, m in enumerate((wq, wk, wv, wp)):
        w4[i] = m.T.astype(bf)
    w4f = w4.reshape(4 * C * C)

    # small f32 buffer (identical on every core)
    small = np.zeros(SLEN, dtype=np.float32)
    small[SP:SP + 256] = tp.astype(np.float32)
    small[SJ:SJ + 256] = tj.astype(np.float32)
    # w9 packed (a, tap, p)
    small[SW9:SW9 + 2304] = (
        w9.reshape(2, 128, 9).transpose(0, 2, 1).astype(np.float32).reshape(-1))

    f8 = ml_dtypes.float8_e4m3
    if "big" not in _CACHE:
        _CACHE["big"] = np.empty((8, XLEN), dtype=bf)
        _CACHE["small"] = np.empty((8, SLEN), dtype=np.float32)
        _CACHE["big8"] = np.empty((8, X8LEN), dtype=f8)
    bigb = _CACHE["big"]
    smallb = _CACHE["small"]
    big8b = _CACHE["big8"]
    smallb[:] = small[None, :]

    q = inputs["q"].astype(f8).reshape(4, C, H, W)
    k = inputs["k"].astype(f8).reshape(4, C, NK)
    v = inputs["v"].astype(f8).reshape(4, C, NK)
    tqb = tq.astype(bf)

    for c in range(8):
        b, half = c // 2, c % 2
        r0 = half * ROWS_HALF
        qx = big8b[c, O8Q:O8Q + NQX].reshape(C, QROWS, W)
        hm = np.zeros((QROWS,), dtype=bf)
        lo, hi = max(0, r0 - 1), min(H, r0 + ROWS_HALF + 1)
        a0 = lo - (r0 - 1)
        if a0 > 0:
            qx[:, 0:a0] = 0
        if a0 + (hi - lo) < QROWS:
            qx[:, a0 + (hi - lo):] = 0
        qx[:, a0:a0 + (hi - lo)] = q[b, :, lo:hi]
        hm[a0:a0 + (hi - lo)] = 1
        big8b[c, O8K:O8K + NKV1] = k[b].reshape(-1)
        big8b[c, O8V:O8V + NKV1] = v[b].reshape(-1)
        bigb[c, OW:OW + NW] = w4f
        bigb[c, OH:OH + NH] = np.repeat(hm, W)
        bigb[c, OT:OT + NT] = tqb

    # stage the packed buffers on-device (sharded over the 8 cores) so
    # repeated run_cores calls don't re-ship inputs over the tunnel
    try:
        dev = _device_put(bigb, smallb, big8b)
    except Exception as e:
        import sys
        print(f"kernel: device_put failed ({type(e).__name__}: {e}); "
              f"staying host-side", file=sys.stderr)
        dev = None
    return bigb, smallb, big8b, dev


def _get_mesh():
    if "mesh" not in _CACHE:
        import jax
        from jax.sharding import Mesh
        _CACHE["mesh"] = Mesh(np.asarray(jax.devices()[:8]), ("core",))
    return _CACHE["mesh"]


def _device_put(bigb, smallb, big8b):
    import jax
    from jax.sharding import NamedSharding, PartitionSpec
    sh = NamedSharding(_get_mesh(), PartitionSpec("core"))
    arrs = (
        jax.device_put(bigb.reshape(-1), sh),
        jax.device_put(smallb.reshape(-1), sh),
        jax.device_put(big8b.reshape(-1), sh),
    )
    jax.block_until_ready(arrs)
    return arrs


def _get_nc():
    if "nc" not in _CACHE:
        _CACHE["nc"] = _build()
    return _CACHE["nc"]


def _get_runner():
    if "runner" in _CACHE:
        return _CACHE["runner"]
    import jax
    from jax.sharding import PartitionSpec
    from jax.experimental.shard_map import shard_map
    from concourse import bass2jax

    nc = _get_nc()
    bass2jax.install_neuronx_cc_hook()
    out_aval = jax.core.ShapedArray((1, C, NQ), ml_dtypes.bfloat16)
    pid_name = nc.partition_id_tensor.name if nc.partition_id_tensor else None
    in_names = ("big", "small", "big8") + ((pid_name,) if pid_name else ())

    def _body(bigv, smallv, big8v):
        operands = [bigv, smallv, big8v]
        if pid_name is not None:
            operands.append(bass2jax.partition_id_tensor())
        outs = bass2jax._bass_exec_p.bind(
            *operands,
            out_avals=(out_aval,),
            in_names=in_names,
            out_names=("o",),
            lowering_input_output_aliases=(),
            sim_require_finite=True,
            sim_require_nnan=True,
            nc=nc,
        )
        return tuple(outs)

    mesh = _get_mesh()
    sharded = jax.jit(
        shard_map(
            _body, mesh=mesh,
            in_specs=(PartitionSpec("core"),) * 3,
            out_specs=(PartitionSpec("core"),),
            check_rep=False,
        ),
        keep_unused=True,
    )
    _CACHE["runner"] = sharded
    return sharded


def _run_fallback(big, small, big8):
    from concourse.bass_utils import run_bass_kernel_spmd
    in_maps = [{"big": big[c], "small": small[c], "big8": big8[c]}
               for c in range(8)]
    res = run_bass_kernel_spmd(_get_nc(), in_maps, core_ids=list(range(8)))
    return np.stack([res.results[c]["o"][0] for c in range(8)])


def run_cores(bufs):
    big, small, big8, dev = bufs
    if "runner_failed" in _CACHE or dev is None:
        return _run_fallback(big, small, big8)
    try:
        sharded = _get_runner()
        out, = sharded(*dev)
        out.block_until_ready()
        return out
    except Exception as e:
        import sys
        print(f"kernel: jit runner failed ({type(e).__name__}: {e}); "
              f"using spmd fallback", file=sys.stderr)
        _CACHE["runner_failed"] = True
        return _run_fallback(big, small, big8)


def assemble(out):
    # out: [8, C, NQ] bf16 (each core's half-batch output shard)
    o8 = np.asarray(out).astype(np.float32).reshape(4, 2, C, ROWS_HALF, W)
    return o8.transpose(0, 2, 1, 3, 4).reshape(4, C, H, W).copy()


def kernel(**inputs):
    bufs = _prep(inputs)
    out = run_cores(bufs)
    return assemble(out)


# revision 19
# speedup vs baseline: 1.0280x; 1.0006x over previous
"""Trainium2 Bass kernel for nn_CrossAttention (B=4, C=256, H=W=48, heads=4).

Sharding: 8 cores = 4 batches x 2 query-row halves. Fully collective-free:
the host stages the folded 1x1-conv weights (replicated) plus the full
raw k/v (fp8) and the core's q row-half (with 1-row halo) per core, so
every core runs an independent local pipeline. fp8 inputs feed the
feature matmuls directly (mixed fp8 x bf16 is supported by the PE).

BN folding tricks: the k-projection bias adds a per-query constant to
every logit row, so softmax cancels it -- dropped. The v-projection bias
adds a per-channel constant to the normalized attention output, so it is
folded into the proj bias on the host (tj' = tj + Wp @ tv).

The positional depthwise 3x3 conv runs on the gpsimd engine (nine
shifted multiply-accumulates), keeping the vector engine free for
PSUM evacuation and softmax normalization; exp runs on the scalar
engine, pipelined across query slices against the PE's S/AV matmuls.
"""

import numpy as np
import ml_dtypes

import concourse.bass as bass
import concourse.mybir as mybir
import concourse.tile as tile
from concourse import bacc

F32 = mybir.dt.float32
BF16 = mybir.dt.bfloat16
F8 = mybir.dt.float8e4

C = 256
H = W = 48
NK = H * W            # 2304 keys
KC = NK // 128        # 18 key chunks
HEADS = 4
HD = 64
ROWS_HALF = 24        # rows per core
NQ = ROWS_HALF * W    # 1152 query positions per core
QS = 384              # query slice (8 rows)
NQS = NQ // QS        # 3 slices
QROWS = ROWS_HALF + 2  # 26 rows incl halo
NQH = QROWS * W       # 1248
EPS = 1e-5

# big (bf16) input layout, element offsets
OW = 0
NW = 4 * C * C                # 262144 (four weight matrices, replicated)
OH = OW + NW
NH = NQH
OT = OH + NH
NT = 2 * 128
XLEN = OT + NT                # 263648

# big8 (fp8) input layout
O8K = 0
NKV1 = C * NK                 # 589824
O8V = O8K + NKV1
O8Q = O8V + NKV1
NQX = C * NQH                 # 319488 (raw q rows incl halo)
X8LEN = O8Q + NQX             # 1499136

# small (f32) input layout: one (a, t, p) block, t = [tp, tj', w9 x9]
# (tj' = tj + Wp @ tv folded)
SLEN = 2 * 11 * 128           # 2816

_CACHE = {}


def _build(iters=1):
    nc = bacc.Bacc("TRN2", target_bir_lowering=False, num_devices=8)
    big = nc.dram_tensor("big", [XLEN], BF16, kind="ExternalInput")
    small = nc.dram_tensor("small", [SLEN], F32, kind="ExternalInput")
    big8 = nc.dram_tensor("big8", [X8LEN], F8, kind="ExternalInput")
    o = nc.dram_tensor("o", [1, C, NQ], BF16, kind="ExternalOutput")

    with tile.TileContext(nc) as tc:
        with (
            tc.tile_pool(name="wp", bufs=1) as wp,
            tc.tile_pool(name="inp", bufs=1) as inp,
            tc.tile_pool(name="feat", bufs=1) as feat,
            tc.tile_pool(name="vfp", bufs=18) as vfp,
            tc.tile_pool(name="et", bufs=40) as etp,
            tc.tile_pool(name="small", bufs=3) as smp,
            tc.tile_pool(name="ys", bufs=4) as ysp,
            tc.tile_pool(name="ps_s", bufs=2, space="PSUM") as ps_s,
            tc.tile_pool(name="ps_w", bufs=2, space="PSUM") as ps_w,
        ):
          for _it in range(iters):
            # ---- inputs to SBUF; big raw tensors and params on separate
            # queues so compute unblocks ASAP (fp8 feeds the PE directly)
            q8 = inp.tile([128, 2, NQH], F8, tag="q8")
            nc.sync.dma_start(
                out=q8[:], in_=big8[O8Q:O8Q + NQX].rearrange("(p a n) -> p a n", a=2, n=NQH))
            k8 = inp.tile([128, 2, NK], F8, tag="k8")
            nc.sync.dma_start(
                out=k8[:], in_=big8[O8K:O8K + NKV1].rearrange("(p a n) -> p a n", a=2, n=NK))
            v8 = inp.tile([128, 2, NK], F8, tag="v8")
            w_all = wp.tile([128, 4, 2, C], BF16, tag="wall")
            nc.scalar.dma_start(
                out=w_all[:],
                in_=big[OW:OW + NW].rearrange("(p w a n) -> p w a n", w=4, a=2, n=C))
            w_q, w_k, w_v, w_p = (w_all[:, i] for i in range(4))

            ht = wp.tile([1, NH + NT], BF16, tag="ht")
            nc.scalar.dma_start(
                out=ht[:], in_=big[OH:OH + NH + NT].rearrange("(x n) -> x n", x=1))
            hq_sb = ht[:, 0:NH]
            tq_sb = ht[:, NH:NH + NT].rearrange("x (a n) -> x a n", a=2)

            prm = wp.tile([128, 2, 11], F32, tag="prm")
            nc.scalar.dma_start(
                out=prm[:], in_=small[0:SLEN].rearrange("(p a t) -> p a t", a=2, t=11))
            tp_sb = prm[:, :, 0:1]
            tj_sb = prm[:, :, 1:2]
            w9_sb = prm[:, :, 2:11]

            # ---- front-end feature emitters (interleaved with slice-0 S) ----
            qf = feat.tile([128, 2, NQH], BF16, tag="qf")
            kf = feat.tile([128, 2, NK], BF16, tag="kf")
            vf = []

            def emit_qf(co, n0):
                # channel-major query features (scaled), with halo rows
                nn = min(512, NQH - n0)
                ps = ps_w.tile([128, 512], F32, tag="w")
                for ci in range(2):
                    nc.tensor.matmul(
                        ps[:, 0:nn],
                        w_q[:, ci, co * 128:(co + 1) * 128],
                        q8[:, ci, n0:n0 + nn],
                        start=(ci == 0), stop=False,
                    )
                # masked bias: qf += tq[c] * hmask[n]  (rank-1)
                nc.tensor.matmul(
                    ps[:, 0:nn],
                    tq_sb[:, co, :],
                    hq_sb[:, n0:n0 + nn],
                    start=False, stop=True,
                )
                nc.vector.tensor_copy(qf[:, co, n0:n0 + nn], ps[:, 0:nn])

            def emit_kf(co, t):
                # key features for one 384-col t-group
                # (no bias: it is constant across keys, softmax cancels it)
                n0 = t * QS
                ps = ps_w.tile([128, 512], F32, tag="w")
                for ci in range(2):
                    nc.tensor.matmul(
                        ps[:, 0:QS],
                        w_k[:, ci, co * 128:(co + 1) * 128],
                        k8[:, ci, n0:n0 + QS],
                        start=(ci == 0), stop=(ci == 1),
                    )
                nc.vector.tensor_copy(kf[:, co, n0:n0 + QS], ps[:, 0:QS])

            def emit_vf(pc):
                # position-major value features [128, 4, 66]
                # per head h: cols [v(64) | 1 | pad]; no bias (folded into tj')
                vt = vfp.tile([128, 4, 66], BF16, tag="vf")
                nc.gpsimd.memset(vt[:, :, 64:66], 1.0)
                ps = ps_w.tile([128, 512], F32, tag="w")
                for ci in range(2):
                    nc.tensor.matmul(
                        ps[:, 0:C],
                        v8[:, ci, pc * 128:(pc + 1) * 128],
                        w_v[:, ci, :],
                        start=(ci == 0), stop=(ci == 1),
                    )
                psv = ps[:, 0:C].rearrange("p (h d) -> p h d", h=4)
                nc.vector.tensor_copy(vt[:, :, 0:64], psv[:])
                vf.append(vt)

            qfr = qf[:].rearrange("p a (r w) -> p a r w", w=W)

            # ---- attention + pe + proj, software-pipelined across q slices:
            # while ACT runs exp for slice si, PE runs AV/pe/proj of si-1.
            def emit_s_group(st, t, h):
                hp, par = h // 2, h % 2
                rs = slice(par * 64, par * 64 + 64)
                s = ps_s.tile([128, 3, 512], F32, tag="s")
                for i in range(3):
                    kc = t * 3 + i
                    nc.tensor.matmul(
                        s[:, i, 0:QS],
                        kf[rs, hp, kc * 128:(kc + 1) * 128],
                        qf[rs, hp, st["q0"]:st["q0"] + QS],
                        start=True, stop=True,
                    )
                et = etp.tile([128, 3, QS], BF16, tag="et")
                nc.scalar.activation(et[:], s[:, :, 0:QS],
                                     mybir.ActivationFunctionType.Exp)
                st["ets"][t][h] = et

            def emit_av_head(st, h):
                y = ps_w.tile([128, 512], F32, tag="w")
                for t in range(6):
                    for i in range(3):
                        kc = t * 3 + i
                        nc.tensor.matmul(
                            y[0:65, 0:QS], vf[kc][:, h, 0:65],
                            st["ets"][t][h][:, i, :],
                            start=(kc == 0), stop=(kc == KC - 1),
                        )
                ysb = ysp.tile([65, QS], F32, tag="ysb")
                nc.vector.tensor_copy(ysb[:], y[0:65, 0:QS])
                st["ys"][h] = ysb

            def emit_norm(st, pair):
                ys = [st["ys"][pair * 2], st["ys"][pair * 2 + 1]]
                ynt = smp.tile([128, QS], BF16, tag="yn")
                rr = smp.tile([1, 2, QS], F32, tag="rr")
                rq = smp.tile([64, 2, QS], F32, tag="rq")
                for par in range(2):
                    nc.vector.reciprocal(rr[:, par, :], ys[par][64:65, :])
                nc.gpsimd.partition_broadcast(rq[:], rr[:])
                nc.vector.tensor_mul(ynt[0:64, :], ys[0][0:64, :], rq[:, 0, :])
                nc.vector.tensor_mul(ynt[64:128, :], ys[1][0:64, :], rq[:, 1, :])
                st["yn"][pair] = ynt

            # pe taps ordered so the first writes the full width (dj == 0)
            PE_TAPS = [(-1, 0), (-1, -1), (-1, 1), (0, -1), (0, 0), (0, 1),
                       (1, -1), (1, 0), (1, 1)]

            def emit_tail(st):
                r0, si = st["r0"], st["si"]
                yt = [None, None]
                for ch in range(2):
                    pet = smp.tile([128, QS], F32, tag="pe")
                    pev = pet[:].rearrange("p (r w) -> p r w", w=W)
                    for idx, (di, dj) in enumerate(PE_TAPS):
                        ti = (di + 1) * 3 + (dj + 1)
                        j0o, j0i = max(0, -dj), max(0, dj)
                        ncol = W - abs(dj)
                        src = qfr[:, ch, r0 + 1 + di:r0 + 9 + di, j0i:j0i + ncol]
                        if idx == 0:
                            nc.vector.tensor_scalar(
                                pev[:, :, j0o:j0o + ncol], src,
                                w9_sb[:, ch, ti:ti + 1], None, mybir.AluOpType.mult,
                            )
                        else:
                            nc.vector.scalar_tensor_tensor(
                                out=pev[:, :, j0o:j0o + ncol], in0=src,
                                scalar=w9_sb[:, ch, ti:ti + 1],
                                in1=pev[:, :, j0o:j0o + ncol],
                                op0=mybir.AluOpType.mult, op1=mybir.AluOpType.add,
                            )
                    ytt = smp.tile([128, QS], BF16, tag="yt")
                    nc.vector.scalar_tensor_tensor(
                        out=ytt[:], in0=pet[:], scalar=tp_sb[:, ch, :],
                        in1=st["yn"][ch][:], op0=mybir.AluOpType.add,
                        op1=mybir.AluOpType.add,
                    )
                    yt[ch] = ytt
                ob = smp.tile([128, 2, QS], BF16, tag="ob")
                for co in range(2):
                    pj = ps_w.tile([128, 512], F32, tag="w")
                    for ci in range(2):
                        nc.tensor.matmul(
                            pj[:, 0:QS],
                            w_p[:, ci, co * 128:(co + 1) * 128],
                            yt[ci][:],
                            start=(ci == 0), stop=(ci == 1),
                        )
                    nc.vector.tensor_scalar(
                        ob[:, co, :], pj[:, 0:QS], tj_sb[:, co, :], None,
                        mybir.AluOpType.add,
                    )
                nc.sync.dma_start(
                    out=o[0].rearrange("(a p) n -> p a n", p=128)[:, :, si * QS:(si + 1) * QS],
                    in_=ob[:],
                )

            FIRE = {4: lambda st: emit_av_head(st, 0),
                    8: lambda st: emit_av_head(st, 1),
                    12: lambda st: emit_norm(st, 0),
                    16: lambda st: emit_av_head(st, 2),
                    20: lambda st: emit_av_head(st, 3),
                    24: lambda st: emit_norm(st, 1)}

            def new_state(si):
                return {"si": si, "q0": 48 + si * QS, "r0": si * (QS // W),
                        "ets": [[None] * HEADS for _ in range(6)],
                        "ys": [None] * 4, "yn": [None, None]}

            # ---- slice 0, interleaved with the front-end: exp starts as
            # soon as the first kf t-group lands (v8 ships during it)
            s0 = new_state(0)
            nc.sync.dma_start(
                out=v8[:], in_=big8[O8V:O8V + NKV1].rearrange("(p a n) -> p a n", a=2, n=NK))
            emit_qf(0, 0)
            emit_qf(1, 0)
            for t in range(6):
                emit_kf(0, t)
                emit_s_group(s0, t, 0)
                emit_s_group(s0, t, 1)
                emit_kf(1, t)
                emit_s_group(s0, t, 2)
                emit_s_group(s0, t, 3)
            for n0 in (512, 1024):
                emit_qf(0, n0)
                emit_qf(1, n0)
            for pc in range(KC):
                emit_vf(pc)

            prev = s0
            for si in range(1, NQS - 1):
                cur = new_state(si)
                g = 0
                for t in range(6):
                    for h in range(HEADS):
                        emit_s_group(cur, t, h)
                        g += 1
                        if g in FIRE:
                            FIRE[g](prev)
                emit_tail(prev)
                prev = cur

            # last slice: h-major S emission with its own AV/norm fired
            # progressively, so the post-exp drain is one AV head deep
            last = new_state(NQS - 1)
            g = 0
            for h in range(HEADS):
                for t in range(6):
                    emit_s_group(last, t, h)
                    g += 1
                    if g in FIRE:
                        FIRE[g](prev)
                if h == 2:
                    emit_av_head(last, 0)
                elif h == 3:
                    emit_av_head(last, 1)
            emit_tail(prev)
            emit_norm(last, 0)
            emit_av_head(last, 2)
            emit_av_head(last, 3)
            emit_norm(last, 1)
            emit_tail(last)
    nc.compile()
    return nc


def _prep(inputs):
    """Host-side: fold BN into weights, pack per-core staged buffers."""
    f64 = np.float64
    bf = ml_dtypes.bfloat16

    def fold(w, g, b, m, v):
        s = g.astype(f64) / np.sqrt(v.astype(f64) + EPS)
        return w.astype(f64) * s[:, None], b.astype(f64) - m.astype(f64) * s

    wq, tq = fold(inputs["wq_w"], inputs["wq_g"], inputs["wq_b"], inputs["wq_m"], inputs["wq_v"])
    wk, _tk = fold(inputs["wk_w"], inputs["wk_g"], inputs["wk_b"], inputs["wk_m"], inputs["wk_v"])
    wv, tv = fold(inputs["wv_w"], inputs["wv_g"], inputs["wv_b"], inputs["wv_m"], inputs["wv_v"])
    wp, tj = fold(inputs["proj_w"], inputs["proj_g"], inputs["proj_b"], inputs["proj_m"], inputs["proj_v"])
    scale = 1.0 / np.sqrt(HD)
    wq, tq = wq * scale, tq * scale
    tj = tj + wp @ tv  # v-projection bias commutes through proj
    s_pe = inputs["pe_g"].astype(f64) / np.sqrt(inputs["pe_v"].astype(f64) + EPS)
    tp = inputs["pe_b"].astype(f64) - inputs["pe_m"].astype(f64) * s_pe
    w9 = inputs["pe_w"].astype(f64).reshape(C, 9) * s_pe[:, None] / scale  # pe sees unscaled qf

    # four weight matrices, transposed, packed partition-major (p, w, a, n)
    w4 = np.empty((4, C, C), dtype=bf)
    for i, m in enumerate((wq, wk, wv, wp)):
        w4[i] = m.T.astype(bf)
    w4f = w4.reshape(4, 2, 128, C).transpose(2, 0, 1, 3).reshape(-1)

    # small f32 buffer (identical on every core): (a, t, p) rows tp|tj|w9
    prm = np.zeros((2, 11, 128), dtype=np.float32)
    prm[:, 0] = tp.astype(np.float32).reshape(2, 128)
    prm[:, 1] = tj.astype(np.float32).reshape(2, 128)
    prm[:, 2:11] = w9.reshape(2, 128, 9).transpose(0, 2, 1).astype(np.float32)
    small = prm.transpose(2, 0, 1).reshape(-1)  # (p, a, t)

    f8 = ml_dtypes.float8_e4m3
    if "big" not in _CACHE:
        _CACHE["big"] = np.empty((8, XLEN), dtype=bf)
        _CACHE["small"] = np.empty((8, SLEN), dtype=np.float32)
        _CACHE["big8"] = np.empty((8, X8LEN), dtype=f8)
    bigb = _CACHE["big"]
    smallb = _CACHE["small"]
    big8b = _CACHE["big8"]
    smallb[:] = small[None, :]

    q = inputs["q"].astype(f8).reshape(4, C, H, W)
    k = inputs["k"].astype(f8).reshape(4, C, NK)
    v = inputs["v"].astype(f8).reshape(4, C, NK)
    tqb = tq.astype(bf)

    for c in range(8):
        b, half = c // 2, c % 2
        r0 = half * ROWS_HALF
        qx = np.zeros((C, QROWS, W), dtype=f8)
        hm = np.zeros((QROWS,), dtype=bf)
        lo, hi = max(0, r0 - 1), min(H, r0 + ROWS_HALF + 1)
        a0 = lo - (r0 - 1)
        qx[:, a0:a0 + (hi - lo)] = q[b, :, lo:hi]
        hm[a0:a0 + (hi - lo)] = 1
        # all raw blocks packed partition-major (p, a, n)
        big8b[c, O8Q:O8Q + NQX] = (
            qx.reshape(2, 128, NQH).transpose(1, 0, 2).reshape(-1))
        big8b[c, O8K:O8K + NKV1] = (
            k[b].reshape(2, 128, NK).transpose(1, 0, 2).reshape(-1))
        big8b[c, O8V:O8V + NKV1] = (
            v[b].reshape(2, 128, NK).transpose(1, 0, 2).reshape(-1))
        bigb[c, OW:OW + NW] = w4f
        bigb[c, OH:OH + NH] = np.repeat(hm, W)
        bigb[c, OT:OT + NT] = tqb

    # stage the packed buffers on-device (sharded over the 8 cores) so
    # repeated run_cores calls don't re-ship inputs over the tunnel
    try:
        dev = _device_put(bigb, smallb, big8b)
    except Exception as e:
        import sys
        print(f"kernel: device_put failed ({type(e).__name__}: {e}); "
              f"staying host-side", file=sys.stderr)
        dev = None
    return bigb, smallb, big8b, dev


def _get_mesh():
    if "mesh" not in _CACHE:
        import jax
        from jax.sharding import Mesh
        _CACHE["mesh"] = Mesh(np.asarray(jax.devices()[:8]), ("core",))
    return _CACHE["mesh"]


def _device_put(bigb, smallb, big8b):
    import jax
    from jax.sharding import NamedSharding, PartitionSpec
    sh = NamedSharding(_get_mesh(), PartitionSpec("core"))
    arrs = (
        jax.device_put(bigb.reshape(-1), sh),
        jax.device_put(smallb.reshape(-1), sh),
        jax.device_put(big8b.reshape(-1), sh),
    )
    jax.block_until_ready(arrs)
    return arrs


def _get_nc():
    if "nc" not in _CACHE:
        _CACHE["nc"] = _build()
    return _CACHE["nc"]


def _get_runner():
    if "runner" in _CACHE:
        return _CACHE["runner"]
    import jax
    from jax.sharding import PartitionSpec
    from jax.experimental.shard_map import shard_map
    from concourse import bass2jax

    nc = _get_nc()
    bass2jax.install_neuronx_cc_hook()
    out_aval = jax.core.ShapedArray((1, C, NQ), ml_dtypes.bfloat16)
    pid_name = nc.partition_id_tensor.name if nc.partition_id_tensor else None
    in_names = ("big", "small", "big8") + ((pid_name,) if pid_name else ())

    def _body(bigv, smallv, big8v):
        operands = [bigv, smallv, big8v]
        if pid_name is not None:
            operands.append(bass2jax.partition_id_tensor())
        outs = bass2jax._bass_exec_p.bind(
            *operands,
            out_avals=(out_aval,),
            in_names=in_names,
            out_names=("o",),
            lowering_input_output_aliases=(),
            sim_require_finite=True,
            sim_require_nnan=True,
            nc=nc,
        )
        return tuple(outs)

    mesh = _get_mesh()
    sharded = jax.jit(
        shard_map(
            _body, mesh=mesh,
            in_specs=(PartitionSpec("core"),) * 3,
            out_specs=(PartitionSpec("core"),),
            check_rep=False,
        ),
        keep_unused=True,
    )
    _CACHE["runner"] = sharded
    return sharded


def _run_fallback(big, small, big8):
    from concourse.bass_utils import run_bass_kernel_spmd
    in_maps = [{"big": big[c], "small": small[c], "big8": big8[c]}
               for c in range(8)]
    res = run_bass_kernel_spmd(_get_nc(), in_maps, core_ids=list(range(8)))
    return np.stack([res.results[c]["o"][0] for c in range(8)])


def run_cores(bufs):
    big, small, big8, dev = bufs
    if "runner_failed" in _CACHE or dev is None:
        return _run_fallback(big, small, big8)
    try:
        sharded = _get_runner()
        out, = sharded(*dev)
        out.block_until_ready()
        return out
    except Exception as e:
        import sys
        print(f"kernel: jit runner failed ({type(e).__name__}: {e}); "
              f"using spmd fallback", file=sys.stderr)
        _CACHE["runner_failed"] = True
        return _run_fallback(big, small, big8)


def assemble(out):
    # out: [8, C, NQ] bf16 (each core's half-batch output shard)
    o8 = np.asarray(out).astype(np.float32).reshape(4, 2, C, ROWS_HALF, W)
    return o8.transpose(0, 2, 1, 3, 4).reshape(4, C, H, W).copy()


def kernel(**inputs):
    bufs = _prep(inputs)
    out = run_cores(bufs)
    return assemble(out)
